# revision 24
# baseline (speedup 1.0000x reference)
"""Brute-force L2 1-NN on 8 TRN2 NeuronCores.

Problem: x [4096, 256], prototypes [32768, 256] -> prototypes[argmin_j ||x-p_j||^2]

Strategy (prototype-sharded SPMD, no collectives):
  - Host sorts the prototype bank by |p|^2 and shards the sorted order across
    8 cores (each core gets a contiguous |p|^2 band); queries replicated.
  - Device computes raw scores s[q, j] = x.p via TensorE fp32r matmuls
    ([q_part, j_free], K=256 as two 128-chunks; fp32r measured at 227 ns per
    128x128x512 matmul warm, abs err <~2e-2 on these magnitudes).
  - ScalarE drains each 4-bank PSUM half to SBUF as bf16; VectorE computes
    per-16-wide-chunk maxes with pairwise tensor_tensor(max) folds (bf16
    all-SBUF -> DVE 2x mode): m[q, g] = max over sorted chunk g of bf16(x.p).
    No positions and no |p|^2 correction on device.
  - Host: for chunk g, the true max of c' = x.p - 0.5|p|^2 lies in
      [m[g] - 0.5 max_psq(g) - eps, m[g] - 0.5 min_psq(g) + eps]
    with eps = fp32r matmul error + bf16 rounding (<= 0.5 ulp, bounded by
    EPS_FP32R). Since chunks are |p|^2-sorted the interval width is tiny;
    interval logic gives an exact-coverage candidate set (~1.3 chunks/query);
    exact float64 rescore of candidate chunks picks the winner; gather rows.

Measured on silicon: ~164-195 us NEFF exec (8 cores SPMD), exact match
against the float32 reference argmin on the target inputs.
"""

import sys
import types

sys.path.insert(0, "/opt/trn_rl_repo")


def _install_ntff_hook():
    try:
        from trn_agent_boot.trn_boot import _ntff_profile_via_ctypes
    except ImportError:
        return
    try:
        hook = _ntff_profile_via_ctypes("/opt/axon/libaxon_pjrt.so")
    except OSError:
        return
    mod = types.ModuleType("antenv.axon_hooks")
    _h = [hook]
    mod.get_axon_ntff_profile_hook = lambda: _h[0]
    mod.set_axon_ntff_profile_hook = lambda h: _h.__setitem__(0, h)
    sys.modules["antenv.axon_hooks"] = mod
    import antenv

    antenv.axon_hooks = mod


_install_ntff_hook()

import numpy as np
import concourse.bass as bass
import concourse.mybir as mybir
import concourse.tile as tile
from concourse import bacc
from concourse.bass_utils import run_bass_kernel_spmd

B, N, D = 4096, 32768, 256
NCORES = 8
NLOC = N // NCORES  # 4096 prototypes per core
QT = 128  # queries per tile
NQT = B // QT  # 32 query tiles
JC = 512  # j-chunk width (one psum bank)
NJC = NLOC // JC  # 8 banks-worth per core
G = 16  # reduce granularity (chunk width for host rescore)
NG = NLOC // G  # 256 chunk maxes per core

# Allowance for |m - exact chunk max|: fp32r matmul error (<~2e-2) plus
# bf16 output rounding (<= 0.5 ulp at |s|<256). Measured max on target
# data: 0.26; 0.60 is a strict upper bound.
EPS_FP32R = 0.60


def build(nqt=NQT, njc=NJC):
    """Build the per-core Bass graph. nqt/njc shrinkable for simulation."""
    f32 = mybir.dt.float32
    f32r = mybir.dt.float32r
    nloc = njc * JC
    b = nqt * QT
    hf = max(1, njc // 2)  # banks per psum half
    ng = nloc // G

    nc = bacc.Bacc("TRN2", target_bir_lowering=False, debug=False, num_devices=NCORES)
    xT_d = nc.dram_tensor("xT", [2, 128, b], f32r, kind="ExternalInput").ap()
    pT_d = nc.dram_tensor("pT", [2, 128, nloc], f32r, kind="ExternalInput").ap()
    m_out = nc.dram_tensor("m", [nqt, QT, ng], mybir.dt.bfloat16, kind="ExternalOutput").ap()

    with tile.TileContext(nc) as tc:
        with (
            tc.tile_pool(name="persist", bufs=1) as pp,
            tc.tile_pool(name="small", bufs=4) as sp,
            tc.tile_pool(name="dbuf", bufs=3) as dbuf,
            tc.tile_pool(name="ps", bufs=2, space="PSUM") as ps,
        ):
            xT_sb = pp.tile([128, 2, b], f32r)
            pT_sb = pp.tile([128, 2, nloc], f32r)
            # split input DMAs, interleaved so the first matmuls (needing
            # xT chunk 0 of both k-planes + pT chunk 0 of both k-planes)
            # can start as early as possible
            for k in range(2):
                nc.sync.dma_start(xT_sb[:, k, bass.ts(0, b // 4)],
                                  xT_d[k][:, bass.ts(0, b // 4)])
            for part in range(njc):
                for k in range(2):
                    sl = bass.ts(part, JC)
                    nc.sync.dma_start(pT_sb[:, k, sl], pT_d[k][:, sl])
            for part in range(1, 4):
                for k in range(2):
                    sl = bass.ts(part, b // 4)
                    nc.sync.dma_start(xT_sb[:, k, sl], xT_d[k][:, sl])

            gph = hf * JC // G  # chunk maxes per half
            bf16 = mybir.dt.bfloat16
            for qt in range(nqt):
                qs = bass.ts(qt, QT)
                nact = max(1, hf - 1)  # banks per half drained via ACT
                nfold = (njc // hf) * nact * JC // G  # fold-path chunk count
                gpb = JC // G  # chunks per bank
                m_sb = sp.tile([QT, ng], bf16, tag="m")
                d_sb = dbuf.tile([QT, nfold, G], bf16, tag="d", name=f"d{qt}")
                for h in range(njc // hf):
                    psum_h = ps.tile([QT, hf, JC], f32, tag="psb", name=f"ps{qt}_{h}")
                    for jc in range(hf):
                        for k in range(2):
                            nc.tensor.matmul(
                                psum_h[:, jc, :],
                                xT_sb[:, k, qs],
                                pT_sb[:, k, bass.ts(h * hf + jc, JC)],
                                start=(k == 0),
                                stop=(k == 1),
                            )
                    # Drain in parallel: DVE reduces the last bank straight
                    # from PSUM while ScalarE copies the first nact banks to
                    # SBUF as bf16. Device m-columns are permuted (fold
                    # chunks first, direct-reduce chunks last); host undoes.
                    if nact < hf:
                        nc.vector.tensor_reduce(
                            m_sb[:, nfold + h * gpb : nfold + (h + 1) * gpb],
                            psum_h[:, nact:hf, :].rearrange(
                                "q c j -> q (c j)"
                            ).rearrange("q (g i) -> q g i", i=G),
                            axis=mybir.AxisListType.X,
                            op=mybir.AluOpType.max,
                        )
                    nc.scalar.copy(
                        d_sb[:, h * nact * gpb : (h + 1) * nact * gpb, :].rearrange(
                            "q g i -> q (g i)"
                        ),
                        psum_h[:, 0:nact, :].rearrange("q c j -> q (c j)"),
                    )
                # VectorE: pairwise-max folds within each 16-group, batched
                # across the whole q-tile (bf16 all-SBUF packed -> DVE 2x)
                f8 = dbuf.tile([QT, nfold, 8], bf16, tag="f8", name=f"f8_{qt}")
                nc.vector.tensor_tensor(
                    out=f8[:], in0=d_sb[:, 0:nfold, 0:8], in1=d_sb[:, 0:nfold, 8:16],
                    op=mybir.AluOpType.max)
                f4 = dbuf.tile([QT, nfold, 4], bf16, tag="f4", name=f"f4_{qt}")
                nc.vector.tensor_tensor(
                    out=f4[:], in0=f8[:, :, 0:4], in1=f8[:, :, 4:8],
                    op=mybir.AluOpType.max)
                f2 = dbuf.tile([QT, nfold, 2], bf16, tag="f2", name=f"f2_{qt}")
                nc.vector.tensor_tensor(
                    out=f2[:], in0=f4[:, :, 0:2], in1=f4[:, :, 2:4],
                    op=mybir.AluOpType.max)
                nc.vector.tensor_tensor(
                    out=m_sb[:, 0:nfold], in0=f2[:, :, 0], in1=f2[:, :, 1],
                    op=mybir.AluOpType.max)
                nc.sync.dma_start(m_out[qt], m_sb[:])
    nc.compile()
    return nc


def _prep_inputs(x, perm_prototypes):
    """Host-side shard prep from the |p|^2-sorted prototype array."""
    xT = np.ascontiguousarray(x.T).reshape(2, 128, B)
    in_maps = []
    for c in range(NCORES):
        P = perm_prototypes[c * NLOC : (c + 1) * NLOC]
        pT = np.ascontiguousarray(P.T).reshape(2, 128, NLOC)
        in_maps.append({"xT": xT, "pT": pT})
    return in_maps


_NC_CACHE = {}


def kernel(x: np.ndarray, prototypes: np.ndarray) -> np.ndarray:
    x = np.asarray(x, dtype=np.float32)
    prototypes = np.asarray(prototypes, dtype=np.float32)
    assert x.shape == (B, D) and prototypes.shape == (N, D)

    if "nc" not in _NC_CACHE:
        _NC_CACHE["nc"] = build()
    nc = _NC_CACHE["nc"]

    # sort prototypes by |p|^2 (host preprocessing / sharding)
    psq = np.einsum("jd,jd->j", prototypes, prototypes)  # fp32
    perm = np.argsort(psq, kind="stable").astype(np.int64)
    P_sorted = prototypes[perm]
    psq_sorted = psq[perm].astype(np.float64)

    in_maps = _prep_inputs(x, P_sorted)
    res = run_bass_kernel_spmd(nc, in_maps, core_ids=list(range(NCORES)))
    _NC_CACHE["last_results"] = res

    # m[c, q, :]: device-permuted chunk maxes -> undo the permutation.
    # Device column layout: fold-path chunks (banks 0..2 of each half) first,
    # then direct-reduce chunks (bank 3 of each half).
    HF = NJC // 2
    NACT = max(1, HF - 1)
    GPB = JC // G
    devcol = np.empty(NG, dtype=np.int64)
    NFOLD = 2 * NACT * GPB
    for g in range(NG):
        b, w = g // GPB, g % GPB
        h, bh = b // HF, b % HF
        if bh < NACT:
            devcol[g] = (h * NACT + bh) * GPB + w
        else:
            devcol[g] = NFOLD + h * GPB + w
    m_all = np.stack(
        [np.asarray(res.results[c]["m"]).astype(np.float32).reshape(B, NG)[:, devcol]
         for c in range(NCORES)]
    )
    m_flat = np.transpose(m_all, (1, 0, 2)).reshape(B, NCORES * NG).astype(np.float64)

    # interval bounds on each chunk's true max of c' = x.p - 0.5 |p|^2
    psq_ch = psq_sorted.reshape(N // G, G)
    hmin = 0.5 * psq_ch.min(axis=1)  # [2048]
    hmax = 0.5 * psq_ch.max(axis=1)
    ub = m_flat - hmin[None, :] + EPS_FP32R
    lb = m_flat - hmax[None, :] - EPS_FP32R
    best_lb = lb.max(axis=1, keepdims=True)
    qs, gs = np.nonzero(ub >= best_lb)  # exact-coverage candidate chunks

    # exact rescore of candidate chunks in float64 (indices in sorted order)
    cand_sj = (gs[:, None] * G + np.arange(G)[None, :]).reshape(-1)
    qq = np.repeat(qs, G)
    cand_j = perm[cand_sj]  # original prototype indices
    pc = prototypes[cand_j].astype(np.float64)
    xc = x[qq].astype(np.float64)
    c_exact = np.einsum("ij,ij->i", pc, xc) - 0.5 * np.einsum("ij,ij->i", pc, pc)
    order = np.lexsort((cand_j, -c_exact, qq))
    qs_o = qq[order]
    first = np.unique(qs_o, return_index=True)[1]
    out_idx = np.empty(B, dtype=np.int64)
    out_idx[qs_o[first]] = cand_j[order][first]

    return prototypes[out_idx]


if __name__ == "__main__":
    rng = np.random.default_rng(0)
    x = rng.standard_normal((B, D), dtype=np.float32)
    p = rng.standard_normal((N, D), dtype=np.float32)
    out = kernel(x, p)
    print("out", out.shape, out.dtype)


# revision 25
# speedup vs baseline: 1.2637x; 1.2637x over previous
"""Brute-force L2 1-NN on 8 TRN2 NeuronCores.

Problem: x [4096, 256], prototypes [32768, 256] -> prototypes[argmin_j ||x-p_j||^2]

Strategy (prototype-sharded SPMD, no collectives):
  - Host sorts the prototype bank by |p|^2 and shards the sorted order across
    8 cores (each core gets a contiguous |p|^2 band); queries replicated.
  - Device computes raw scores s[q, j] = x.p via TensorE fp32r matmuls
    ([q_part, j_free], K=256 as two 128-chunks; fp32r measured at 227 ns per
    128x128x512 matmul warm, abs err <~2e-2 on these magnitudes).
  - ScalarE drains each 4-bank PSUM half to SBUF as bf16; VectorE computes
    per-16-wide-chunk maxes with pairwise tensor_tensor(max) folds (bf16
    all-SBUF -> DVE 2x mode): m[q, g] = max over sorted chunk g of bf16(x.p).
    No positions and no |p|^2 correction on device.
  - Host: for chunk g, the true max of c' = x.p - 0.5|p|^2 lies in
      [m[g] - 0.5 max_psq(g) - eps, m[g] - 0.5 min_psq(g) + eps]
    with eps = fp32r matmul error + bf16 rounding (<= 0.5 ulp, bounded by
    EPS_FP32R). Since chunks are |p|^2-sorted the interval width is tiny;
    interval logic gives an exact-coverage candidate set (~1.3 chunks/query);
    exact float64 rescore of candidate chunks picks the winner; gather rows.

Measured on silicon: ~164-195 us NEFF exec (8 cores SPMD), exact match
against the float32 reference argmin on the target inputs.
"""

import sys
import types

sys.path.insert(0, "/opt/trn_rl_repo")


def _install_ntff_hook():
    try:
        from trn_agent_boot.trn_boot import _ntff_profile_via_ctypes
    except ImportError:
        return
    try:
        hook = _ntff_profile_via_ctypes("/opt/axon/libaxon_pjrt.so")
    except OSError:
        return
    mod = types.ModuleType("antenv.axon_hooks")
    _h = [hook]
    mod.get_axon_ntff_profile_hook = lambda: _h[0]
    mod.set_axon_ntff_profile_hook = lambda h: _h.__setitem__(0, h)
    sys.modules["antenv.axon_hooks"] = mod
    import antenv

    antenv.axon_hooks = mod


_install_ntff_hook()

import numpy as np
import concourse.bass as bass
import concourse.mybir as mybir
import concourse.tile as tile
from concourse import bacc
from concourse.bass_utils import run_bass_kernel_spmd

B, N, D = 4096, 32768, 256
NCORES = 8
NLOC = N // NCORES  # 4096 prototypes per core
QT = 128  # queries per tile
NQT = B // QT  # 32 query tiles
JC = 512  # j-chunk width (one psum bank)
NJC = NLOC // JC  # 8 banks-worth per core
G = 16  # reduce granularity (chunk width for host rescore)
NG = NLOC // G  # 256 chunk maxes per core

# Allowance for |m - exact chunk max|: fp32r matmul error (<~2e-2) plus
# bf16 output rounding (<= 0.5 ulp at |s|<256). Measured max on target
# data: 0.26; 0.60 is a strict upper bound.
EPS_FP32R = 0.60


def build(nqt=NQT, njc=NJC):
    """Build the per-core Bass graph. nqt/njc shrinkable for simulation."""
    f32 = mybir.dt.float32
    f32r = mybir.dt.float32r
    nloc = njc * JC
    b = nqt * QT
    hf = max(1, njc // 2)  # banks per psum half
    ng = nloc // G

    nc = bacc.Bacc("TRN2", target_bir_lowering=False, debug=False, num_devices=NCORES)
    xT_d = nc.dram_tensor("xT", [2, 128, b], f32r, kind="ExternalInput").ap()
    pT_d = nc.dram_tensor("pT", [2, 128, nloc], f32r, kind="ExternalInput").ap()
    m_out = nc.dram_tensor("m", [nqt, QT, ng], mybir.dt.bfloat16, kind="ExternalOutput").ap()

    with tile.TileContext(nc) as tc:
        with (
            tc.tile_pool(name="persist", bufs=1) as pp,
            tc.tile_pool(name="small", bufs=4) as sp,
            tc.tile_pool(name="dbuf", bufs=3) as dbuf,
            tc.tile_pool(name="ps", bufs=2, space="PSUM") as ps,
        ):
            xT_sb = pp.tile([128, 2, b], f32r)
            pT_sb = pp.tile([128, 2, nloc], f32r)
            # split input DMAs, interleaved so the first matmuls (needing
            # xT chunk 0 of both k-planes + pT chunk 0 of both k-planes)
            # can start as early as possible
            for k in range(2):
                nc.sync.dma_start(xT_sb[:, k, bass.ts(0, b // 4)],
                                  xT_d[k][:, bass.ts(0, b // 4)])
            for part in range(njc):
                for k in range(2):
                    sl = bass.ts(part, JC)
                    nc.sync.dma_start(pT_sb[:, k, sl], pT_d[k][:, sl])
            for part in range(1, 4):
                for k in range(2):
                    sl = bass.ts(part, b // 4)
                    nc.sync.dma_start(xT_sb[:, k, sl], xT_d[k][:, sl])

            gph = hf * JC // G  # chunk maxes per half
            bf16 = mybir.dt.bfloat16
            for qt in range(nqt):
                qs = bass.ts(qt, QT)
                m_sb = sp.tile([QT, ng], bf16, tag="m")
                d_sb = dbuf.tile([QT, ng, G], bf16, tag="d", name=f"d{qt}")
                for h in range(njc // hf):
                    psum_h = ps.tile([QT, hf, JC], f32, tag="psb", name=f"ps{qt}_{h}")
                    for jc in range(hf):
                        for k in range(2):
                            nc.tensor.matmul(
                                psum_h[:, jc, :],
                                xT_sb[:, k, qs],
                                pT_sb[:, k, bass.ts(h * hf + jc, JC)],
                                start=(k == 0),
                                stop=(k == 1),
                            )
                    # ScalarE: PSUM -> SBUF, converting to bf16 (drains psum)
                    nc.scalar.copy(
                        d_sb[:, h * gph : (h + 1) * gph, :].rearrange(
                            "q g i -> q (g i)"
                        ),
                        psum_h[:].rearrange("q c j -> q (c j)"),
                    )
                # VectorE: pairwise-max folds within each 16-group, batched
                # across the whole q-tile (bf16 all-SBUF packed -> DVE 2x)
                f8 = dbuf.tile([QT, ng, 8], bf16, tag="f8", name=f"f8_{qt}")
                nc.vector.tensor_tensor(
                    out=f8[:], in0=d_sb[:, :, 0:8], in1=d_sb[:, :, 8:16],
                    op=mybir.AluOpType.max)
                f4 = dbuf.tile([QT, ng, 4], bf16, tag="f4", name=f"f4_{qt}")
                nc.vector.tensor_tensor(
                    out=f4[:], in0=f8[:, :, 0:4], in1=f8[:, :, 4:8],
                    op=mybir.AluOpType.max)
                f2 = dbuf.tile([QT, ng, 2], bf16, tag="f2", name=f"f2_{qt}")
                nc.vector.tensor_tensor(
                    out=f2[:], in0=f4[:, :, 0:2], in1=f4[:, :, 2:4],
                    op=mybir.AluOpType.max)
                nc.vector.tensor_tensor(
                    out=m_sb[:], in0=f2[:, :, 0], in1=f2[:, :, 1],
                    op=mybir.AluOpType.max)
                nc.sync.dma_start(m_out[qt], m_sb[:])
    nc.compile()
    return nc


def _prep_inputs(x, perm_prototypes):
    """Host-side shard prep from the |p|^2-sorted prototype array."""
    xT = np.ascontiguousarray(x.T).reshape(2, 128, B)
    in_maps = []
    for c in range(NCORES):
        P = perm_prototypes[c * NLOC : (c + 1) * NLOC]
        pT = np.ascontiguousarray(P.T).reshape(2, 128, NLOC)
        in_maps.append({"xT": xT, "pT": pT})
    return in_maps


_NC_CACHE = {}


def kernel(x: np.ndarray, prototypes: np.ndarray) -> np.ndarray:
    x = np.asarray(x, dtype=np.float32)
    prototypes = np.asarray(prototypes, dtype=np.float32)
    assert x.shape == (B, D) and prototypes.shape == (N, D)

    if "nc" not in _NC_CACHE:
        _NC_CACHE["nc"] = build()
    nc = _NC_CACHE["nc"]

    # sort prototypes by |p|^2 (host preprocessing / sharding)
    psq = np.einsum("jd,jd->j", prototypes, prototypes)  # fp32
    perm = np.argsort(psq, kind="stable").astype(np.int64)
    P_sorted = prototypes[perm]
    psq_sorted = psq[perm].astype(np.float64)

    in_maps = _prep_inputs(x, P_sorted)
    res = run_bass_kernel_spmd(nc, in_maps, core_ids=list(range(NCORES)))
    _NC_CACHE["last_results"] = res

    # m[c, q, g]: max of x.p over sorted 16-chunk g of core c (fp32r-accurate)
    m_all = np.stack(
        [np.asarray(res.results[c]["m"]).astype(np.float32).reshape(B, NG)
         for c in range(NCORES)]
    )
    m_flat = np.transpose(m_all, (1, 0, 2)).reshape(B, NCORES * NG).astype(np.float64)

    # interval bounds on each chunk's true max of c' = x.p - 0.5 |p|^2
    psq_ch = psq_sorted.reshape(N // G, G)
    hmin = 0.5 * psq_ch.min(axis=1)  # [2048]
    hmax = 0.5 * psq_ch.max(axis=1)
    ub = m_flat - hmin[None, :] + EPS_FP32R
    lb = m_flat - hmax[None, :] - EPS_FP32R
    best_lb = lb.max(axis=1, keepdims=True)
    qs, gs = np.nonzero(ub >= best_lb)  # exact-coverage candidate chunks

    # exact rescore of candidate chunks in float64 (indices in sorted order)
    cand_sj = (gs[:, None] * G + np.arange(G)[None, :]).reshape(-1)
    qq = np.repeat(qs, G)
    cand_j = perm[cand_sj]  # original prototype indices
    pc = prototypes[cand_j].astype(np.float64)
    xc = x[qq].astype(np.float64)
    c_exact = np.einsum("ij,ij->i", pc, xc) - 0.5 * np.einsum("ij,ij->i", pc, pc)
    order = np.lexsort((cand_j, -c_exact, qq))
    qs_o = qq[order]
    first = np.unique(qs_o, return_index=True)[1]
    out_idx = np.empty(B, dtype=np.int64)
    out_idx[qs_o[first]] = cand_j[order][first]

    return prototypes[out_idx]


if __name__ == "__main__":
    rng = np.random.default_rng(0)
    x = rng.standard_normal((B, D), dtype=np.float32)
    p = rng.standard_normal((N, D), dtype=np.float32)
    out = kernel(x, p)
    print("out", out.shape, out.dtype)
